# revision 9
# baseline (speedup 1.0000x reference)
"""Trainium2 kernel for nn_ABlock_48000554500568.

Data-parallel over 8 NeuronCores: one batch sample per core.

The device executes the FLOP-dominant ASM-propagation block per sample:
  U0 -> fft2 -> multiply by P (freq gain + propagation phase, 1/N^2 folded)
     -> ifft2 -> multiply by per-pixel phase correction -> J = |Uz|
as bf16 TensorEngine matmuls (dense 512-point DFTs). The 2D DFT
Y = F X F (F symmetric) is computed with two "data-as-lhsT" matmul
passes, which lands every intermediate in the natural layout with no
explicit transposes.

The small CNN heads (phase/z heads, mix head, SE) run on host CPU jax.
"""

import numpy as np
import ml_dtypes  # noqa: F401  (bf16 numpy dtype registration)

import concourse.bass as bass
import concourse.tile as tile
from concourse import mybir, bacc
from concourse.bass_utils import run_bass_kernel_spmd

# ---------------------------------------------------------------- constants
Z_MAX = 0.3
WAVELENGTHS = np.array([0.65, 0.53, 0.47], np.float32)
LUMA = np.array([0.299, 0.587, 0.114], np.float32)
H = W = 512
C = 3
NCORES = 8


# ------------------------------------------------- host math (pure numpy)
def _sigmoid(x):
    return 1.0 / (1.0 + np.exp(-x))


def _silu(x):
    return x * _sigmoid(x)


def _conv2d(x, w, b):
    # x (B,C,H,W) f32, w (O,C,kh,kw), SAME padding stride 1
    x = np.asarray(x, np.float32)
    w = np.asarray(w, np.float32)
    kh, kw = w.shape[2], w.shape[3]
    ph, pw = kh // 2, kw // 2
    B, Cc, Hh, Ww = x.shape
    O = w.shape[0]
    if kh == kw == 1:
        y = np.einsum("oc,bchw->bohw", w[:, :, 0, 0], x, optimize=True)
    else:
        xp = np.pad(x, ((0, 0), (0, 0), (ph, ph), (pw, pw)))
        y = np.zeros((B, O, Hh, Ww), np.float32)
        for dy in range(kh):
            for dx in range(kw):
                y += np.einsum("oc,bchw->bohw", w[:, :, dy, dx],
                               xp[:, :, dy:dy + Hh, dx:dx + Ww],
                               optimize=True)
    return y + np.asarray(b, np.float32)[None, :, None, None]


def _group_norm(x, g, b, eps=1e-5):
    mu = x.mean(axis=(1, 2, 3), keepdims=True, dtype=np.float64)
    var = ((x - mu) ** 2).mean(axis=(1, 2, 3), keepdims=True, dtype=np.float64)
    xn = (x - mu) / np.sqrt(var + eps)
    return (xn * np.asarray(g, np.float32)[None, :, None, None]
            + np.asarray(b, np.float32)[None, :, None, None]).astype(np.float32)


def _host_pre(x, norm_g, norm_b, ph_w1, ph_b1, ph_w2, ph_b2,
              z_w1, z_b1, z_w2, z_b2):
    """GroupNorm + phase/z heads -> U0, z, z_mean."""
    xn = _group_norm(x, norm_g, norm_b)
    h = _silu(_conv2d(xn, ph_w1, ph_b1))
    phi = np.tanh(_conv2d(h, ph_w2, ph_b2)) * np.float32(np.pi)
    hz = _silu(_conv2d(xn, z_w1, z_b1))
    z = _sigmoid(_conv2d(hz, z_w2, z_b2)) * np.float32(Z_MAX)
    u0r = x * np.cos(phi)
    u0i = x * np.sin(phi)
    z_mean = z.mean(axis=(2, 3), keepdims=True)
    return (u0r.astype(np.float32), u0i.astype(np.float32),
            z.astype(np.float32), z_mean.astype(np.float32))


def _host_post(x, J, mix_w1, mix_b1, gn1_g, gn1_b, mix_w2, mix_b2,
               gn2_g, gn2_b, mix_w3, mix_b3, se_w1, se_b1, se_w2, se_b2,
               alpha):
    """Mix head + SE + residual."""
    lw = LUMA[None, :, None, None]
    x_l = (x * lw).sum(axis=1, keepdims=True)
    J_l = (J * lw).sum(axis=1, keepdims=True)
    mix_in = np.concatenate([x, J_l, J_l - x_l], axis=1).astype(np.float32)
    d = _silu(_group_norm(_conv2d(mix_in, mix_w1, mix_b1), gn1_g, gn1_b))
    d = _silu(_group_norm(_conv2d(d, mix_w2, mix_b2), gn2_g, gn2_b))
    delta = _conv2d(d, mix_w3, mix_b3)
    p = delta.mean(axis=(2, 3))
    wse = _sigmoid(
        _silu(p @ np.asarray(se_w1).T + se_b1) @ np.asarray(se_w2).T + se_b2)
    delta = delta * wse[:, :, None, None]
    return (x + np.float32(alpha) * delta).astype(np.float32)


# ------------------------------------------------------------- bass kernel
_KERNEL_CACHE = {}


def _build_fft_kernel():
    """Per-core kernel: (u0r,u0i,pr,pi,ccp,scp)[3,512,512] f32 -> jout."""
    if "nc" in _KERNEL_CACHE:
        return _KERNEL_CACHE["nc"]

    nc = bacc.Bacc("TRN2", target_bir_lowering=False, debug=False,
                   num_devices=NCORES)
    f32, bf16 = mybir.dt.float32, mybir.dt.bfloat16

    ins = {}
    for name in ("u0r", "u0i", "pr", "pi", "ccp", "scp"):
        ins[name] = nc.dram_tensor(name, [C, H, W], f32, kind="ExternalInput")
    jout = nc.dram_tensor("jout", [C, H, W], f32, kind="ExternalOutput")

    # DFT matrix F[h,v] = exp(-2i*pi*h*v/N), symmetric.
    idx = np.arange(H, dtype=np.float64)
    ang = -2.0 * np.pi * np.outer(idx, idx) / H
    Fr_np = np.cos(ang).astype(np.float32)
    Fi_np = np.sin(ang).astype(np.float32)

    def chunked(a):  # [512,512] -> [128, 4, 512] with h = 128*j + p
        return np.ascontiguousarray(
            a.reshape(4, 128, W).transpose(1, 0, 2)).astype(ml_dtypes.bfloat16)

    fr_d = nc.inline_tensor(chunked(Fr_np), name="Fr")
    fi_d = nc.inline_tensor(chunked(Fi_np), name="Fi")
    fn_d = nc.inline_tensor(chunked(-Fi_np), name="Fn")

    with tile.TileContext(nc) as tc:
        with (
            tc.tile_pool(name="consts", bufs=1) as consts,
            tc.tile_pool(name="stage", bufs=2) as stage,
            tc.tile_pool(name="plane", bufs=2) as plane,
            tc.tile_pool(name="mid", bufs=1) as mid,
            tc.tile_pool(name="psum", bufs=4, space="PSUM") as psum,
            tc.tile_pool(name="outp", bufs=2) as outp,
        ):
            fr = consts.tile([128, 4, W], bf16)
            fi = consts.tile([128, 4, W], bf16)
            fn = consts.tile([128, 4, W], bf16)
            nc.sync.dma_start(fr[:], fr_d.ap().rearrange("p j w -> p (j w)"))
            nc.sync.dma_start(fi[:], fi_d.ap().rearrange("p j w -> p (j w)"))
            nc.sync.dma_start(fn[:], fn_d.ap().rearrange("p j w -> p (j w)"))
            eps_t = consts.tile([128, 1], f32)
            nc.vector.memset(eps_t[:], 1e-12)

            def load_plane_bf16(dram, c, tag):
                """DMA one [512,512] f32 plane -> bf16 [128,4,512] tile."""
                t32 = stage.tile([128, 4, W], f32, tag="stage32")
                nc.sync.dma_start(
                    t32[:], dram.ap()[c].rearrange("(j p) w -> p j w", p=128))
                t16 = plane.tile([128, 4, W], bf16, tag=tag)
                nc.vector.tensor_copy(t16[:], t32[:])
                return t16

            def dft_pass(ar, ai, rr, ri, rn, tag):
                """out[a,b] = sum_k A[k,a] * R[k,b]  (A = data as lhsT).

                A given as (ar, ai) bf16 [128,4,512]; R = complex rhs with
                components (rr for real-part combo r1, ...) — caller passes
                the four (lhsT, rhs) accumulation pairs explicitly.
                Returns (or_, oi_) bf16 [128,4,512].
                """
                o_r = mid.tile([128, 4, W], bf16, tag=tag + "r")
                o_i = mid.tile([128, 4, W], bf16, tag=tag + "i")
                for m in range(4):
                    ps_r = psum.tile([128, W], f32, tag="psr")
                    ps_i = psum.tile([128, W], f32, tag="psi")
                    for k in range(4):
                        first, last = (k == 0), (k == 3)
                        nc.tensor.matmul(
                            ps_r[:], ar[:, k, bass.ts(m, 128)], rr[:, k, :],
                            start=first, stop=False)
                        nc.tensor.matmul(
                            ps_r[:], ai[:, k, bass.ts(m, 128)], rn[:, k, :],
                            start=False, stop=last)
                        nc.tensor.matmul(
                            ps_i[:], ar[:, k, bass.ts(m, 128)], ri[:, k, :],
                            start=first, stop=False)
                        nc.tensor.matmul(
                            ps_i[:], ai[:, k, bass.ts(m, 128)], rr[:, k, :],
                            start=False, stop=last)
                    nc.any.tensor_copy(o_r[:, m, :], ps_r[:])
                    nc.any.tensor_copy(o_i[:, m, :], ps_i[:])
                return o_r, o_i

            def cmul(ar, ai, br, bi, tag):
                """(ar+i*ai) * (br+i*bi) elementwise -> bf16 tiles."""
                t1 = mid.tile([128, 4, W], f32, tag="cm_t1")
                t2 = mid.tile([128, 4, W], f32, tag="cm_t2")
                o_r = plane.tile([128, 4, W], bf16, tag=tag + "r")
                o_i = plane.tile([128, 4, W], bf16, tag=tag + "i")
                nc.vector.tensor_mul(t1[:], ar[:], br[:])
                nc.vector.tensor_mul(t2[:], ai[:], bi[:])
                nc.vector.tensor_sub(o_r[:], t1[:], t2[:])
                nc.vector.tensor_mul(t1[:], ar[:], bi[:])
                nc.vector.tensor_mul(t2[:], ai[:], br[:])
                nc.vector.tensor_add(o_i[:], t1[:], t2[:])
                return o_r, o_i

            for c in range(C):
                xr = load_plane_bf16(ins["u0r"], c, "xr")
                xi = load_plane_bf16(ins["u0i"], c, "xi")

                # forward fft2: two passes against F = Fr + i*Fi
                t1r, t1i = dft_pass(xr, xi, fr, fi, fn, "t1")
                yr, yi = dft_pass(t1r, t1i, fr, fi, fn, "y")

                # multiply by P (includes (1+freq_gain), exp(i kz zm), 1/N^2)
                p_r = load_plane_bf16(ins["pr"], c, "pr")
                p_i = load_plane_bf16(ins["pi"], c, "pi")
                gr, gi = cmul(yr, yi, p_r, p_i, "g")

                # inverse fft2 (unnormalized): two passes against conj(F):
                # real combo: Gr*Fr + Gi*Fi ; imag combo: Gi*Fr - Gr*Fi
                def idft_pass(ar, ai, tag):
                    o_r = mid.tile([128, 4, W], bf16, tag=tag + "r")
                    o_i = mid.tile([128, 4, W], bf16, tag=tag + "i")
                    for m in range(4):
                        ps_r = psum.tile([128, W], f32, tag="psr")
                        ps_i = psum.tile([128, W], f32, tag="psi")
                        for k in range(4):
                            first, last = (k == 0), (k == 3)
                            nc.tensor.matmul(
                                ps_r[:], ar[:, k, bass.ts(m, 128)],
                                fr[:, k, :], start=first, stop=False)
                            nc.tensor.matmul(
                                ps_r[:], ai[:, k, bass.ts(m, 128)],
                                fi[:, k, :], start=False, stop=last)
                            nc.tensor.matmul(
                                ps_i[:], ar[:, k, bass.ts(m, 128)],
                                fn[:, k, :], start=first, stop=False)
                            nc.tensor.matmul(
                                ps_i[:], ai[:, k, bass.ts(m, 128)],
                                fr[:, k, :], start=False, stop=last)
                        nc.any.tensor_copy(o_r[:, m, :], ps_r[:])
                        nc.any.tensor_copy(o_i[:, m, :], ps_i[:])
                    return o_r, o_i

                t3r, t3i = idft_pass(gr, gi, "t3")
                uzr, uzi = idft_pass(t3r, t3i, "uz")

                # phase correction exp(i cp): (uzr+i uzi)*(ccp+i scp)
                c_r = load_plane_bf16(ins["ccp"], c, "ccp")
                c_i = load_plane_bf16(ins["scp"], c, "scp")
                wr, wi = cmul(uzr, uzi, c_r, c_i, "w")

                # J = sqrt(wr^2 + wi^2 + 1e-12)
                sq1 = mid.tile([128, 4, W], f32, tag="sq1")
                sq2 = mid.tile([128, 4, W], f32, tag="sq2")
                nc.vector.tensor_mul(sq1[:], wr[:], wr[:])
                nc.vector.tensor_mul(sq2[:], wi[:], wi[:])
                nc.vector.tensor_add(sq1[:], sq1[:], sq2[:])
                jt = outp.tile([128, 4, W], f32, tag="j")
                nc.scalar.activation(jt[:], sq1[:],
                                     mybir.ActivationFunctionType.Sqrt,
                                     bias=eps_t[:], scale=1.0)
                nc.sync.dma_start(
                    jout.ap()[c].rearrange("(j p) w -> p j w", p=128), jt[:])

    nc.compile()
    _KERNEL_CACHE["nc"] = nc
    return nc


# ------------------------------------------------------------------ kernel
def kernel(**inputs):
    x = np.asarray(inputs["x"], np.float32)
    B = x.shape[0]

    u0r, u0i, z, z_mean = _host_pre(
        x, inputs["norm_g"], inputs["norm_b"],
        inputs["ph_w1"], inputs["ph_b1"], inputs["ph_w2"], inputs["ph_b2"],
        inputs["z_w1"], inputs["z_b1"], inputs["z_w2"], inputs["z_b2"])

    # frequency-domain multiplier P = (1+g)/N^2 * exp(i kz z_mean)
    fy = np.fft.fftfreq(H).astype(np.float32)
    fx = np.fft.fftfreq(W).astype(np.float32)
    f2 = fy[:, None] ** 2 + fx[None, :] ** 2
    inv_l2 = (1.0 / WAVELENGTHS ** 2)[:, None, None]
    kz = 2.0 * np.pi * np.sqrt(np.maximum(inv_l2 - f2[None], 0.0))  # (3,H,W)
    gain = (1.0 + np.asarray(inputs["freq_gain"], np.float32))[None, :, None, None]
    hp = kz[None] * z_mean[:, :, :, :]                # (B,3,H,W)
    scale = gain / (H * W)
    pr = (scale * np.cos(hp)).astype(np.float32)
    pi = (scale * np.sin(hp)).astype(np.float32)

    # spatial phase correction exp(i k0 (z - z_mean))
    k0 = (2.0 * np.pi / WAVELENGTHS)[None, :, None, None]
    cp = k0 * (z - z_mean)
    ccp = np.cos(cp).astype(np.float32)
    scp = np.sin(cp).astype(np.float32)

    nc = _build_fft_kernel()
    in_maps = []
    for b in range(NCORES):
        bb = min(b, B - 1)
        in_maps.append({
            "u0r": np.ascontiguousarray(u0r[bb]),
            "u0i": np.ascontiguousarray(u0i[bb]),
            "pr": np.ascontiguousarray(pr[bb]),
            "pi": np.ascontiguousarray(pi[bb]),
            "ccp": np.ascontiguousarray(ccp[bb]),
            "scp": np.ascontiguousarray(scp[bb]),
        })
    global _LAST_IN_MAPS
    _LAST_IN_MAPS = in_maps
    res = run_bass_kernel_spmd(nc, in_maps, core_ids=list(range(NCORES)))
    J = np.stack([res.results[b]["jout"] for b in range(B)], axis=0)

    out = _host_post(
        x, J.astype(np.float32),
        inputs["mix_w1"], inputs["mix_b1"], inputs["gn1_g"], inputs["gn1_b"],
        inputs["mix_w2"], inputs["mix_b2"], inputs["gn2_g"], inputs["gn2_b"],
        inputs["mix_w3"], inputs["mix_b3"],
        inputs["se_w1"], inputs["se_b1"], inputs["se_w2"], inputs["se_b2"],
        np.float32(inputs["alpha"]))
    return np.asarray(out, np.float32)


# revision 12
# speedup vs baseline: 1.1052x; 1.1052x over previous
"""Trainium2 kernel for nn_ABlock_48000554500568.

Data-parallel over 8 NeuronCores: one batch sample per core.

The device executes the FLOP-dominant ASM-propagation block per sample:
  U0 -> fft2 -> multiply by P (freq gain + propagation phase, 1/N^2 folded)
     -> ifft2 -> multiply by per-pixel phase correction -> J = |Uz|
as bf16 TensorEngine matmuls (dense 512-point DFTs). The 2D DFT
Y = F X F (F symmetric) is computed with two "data-as-lhsT" matmul
passes, which lands every intermediate in the natural layout with no
explicit transposes.

The small CNN heads (phase/z heads, mix head, SE) run on host CPU jax.
"""

import numpy as np
import ml_dtypes  # noqa: F401  (bf16 numpy dtype registration)

import concourse.bass as bass
import concourse.tile as tile
from concourse import mybir, bacc
from concourse.bass_utils import run_bass_kernel_spmd

# ---------------------------------------------------------------- constants
Z_MAX = 0.3
WAVELENGTHS = np.array([0.65, 0.53, 0.47], np.float32)
LUMA = np.array([0.299, 0.587, 0.114], np.float32)
H = W = 512
C = 3
NCORES = 8


# ------------------------------------------------- host math (pure numpy)
def _sigmoid(x):
    return 1.0 / (1.0 + np.exp(-x))


def _silu(x):
    return x * _sigmoid(x)


def _conv2d(x, w, b):
    # x (B,C,H,W) f32, w (O,C,kh,kw), SAME padding stride 1
    x = np.asarray(x, np.float32)
    w = np.asarray(w, np.float32)
    kh, kw = w.shape[2], w.shape[3]
    ph, pw = kh // 2, kw // 2
    B, Cc, Hh, Ww = x.shape
    O = w.shape[0]
    if kh == kw == 1:
        y = np.einsum("oc,bchw->bohw", w[:, :, 0, 0], x, optimize=True)
    else:
        xp = np.pad(x, ((0, 0), (0, 0), (ph, ph), (pw, pw)))
        y = np.zeros((B, O, Hh, Ww), np.float32)
        for dy in range(kh):
            for dx in range(kw):
                y += np.einsum("oc,bchw->bohw", w[:, :, dy, dx],
                               xp[:, :, dy:dy + Hh, dx:dx + Ww],
                               optimize=True)
    return y + np.asarray(b, np.float32)[None, :, None, None]


def _group_norm(x, g, b, eps=1e-5):
    mu = x.mean(axis=(1, 2, 3), keepdims=True, dtype=np.float64)
    var = ((x - mu) ** 2).mean(axis=(1, 2, 3), keepdims=True, dtype=np.float64)
    xn = (x - mu) / np.sqrt(var + eps)
    return (xn * np.asarray(g, np.float32)[None, :, None, None]
            + np.asarray(b, np.float32)[None, :, None, None]).astype(np.float32)


def _host_pre(x, norm_g, norm_b, ph_w1, ph_b1, ph_w2, ph_b2,
              z_w1, z_b1, z_w2, z_b2):
    """GroupNorm + phase/z heads -> U0, z, z_mean."""
    xn = _group_norm(x, norm_g, norm_b)
    h = _silu(_conv2d(xn, ph_w1, ph_b1))
    phi = np.tanh(_conv2d(h, ph_w2, ph_b2)) * np.float32(np.pi)
    hz = _silu(_conv2d(xn, z_w1, z_b1))
    z = _sigmoid(_conv2d(hz, z_w2, z_b2)) * np.float32(Z_MAX)
    u0r = x * np.cos(phi)
    u0i = x * np.sin(phi)
    z_mean = z.mean(axis=(2, 3), keepdims=True)
    return (u0r.astype(np.float32), u0i.astype(np.float32),
            z.astype(np.float32), z_mean.astype(np.float32))


def _host_post(x, J, mix_w1, mix_b1, gn1_g, gn1_b, mix_w2, mix_b2,
               gn2_g, gn2_b, mix_w3, mix_b3, se_w1, se_b1, se_w2, se_b2,
               alpha):
    """Mix head + SE + residual."""
    lw = LUMA[None, :, None, None]
    x_l = (x * lw).sum(axis=1, keepdims=True)
    J_l = (J * lw).sum(axis=1, keepdims=True)
    mix_in = np.concatenate([x, J_l, J_l - x_l], axis=1).astype(np.float32)
    d = _silu(_group_norm(_conv2d(mix_in, mix_w1, mix_b1), gn1_g, gn1_b))
    d = _silu(_group_norm(_conv2d(d, mix_w2, mix_b2), gn2_g, gn2_b))
    delta = _conv2d(d, mix_w3, mix_b3)
    p = delta.mean(axis=(2, 3))
    wse = _sigmoid(
        _silu(p @ np.asarray(se_w1).T + se_b1) @ np.asarray(se_w2).T + se_b2)
    delta = delta * wse[:, :, None, None]
    return (x + np.float32(alpha) * delta).astype(np.float32)


# ------------------------------------------------------------- bass kernel
_KERNEL_CACHE = {}


def _build_fft_kernel():
    """Per-core kernel: (u0r,u0i,pr,pi,ccp,scp)[3,512,512] f32 -> jout."""
    if "nc" in _KERNEL_CACHE:
        return _KERNEL_CACHE["nc"]

    nc = bacc.Bacc("TRN2", target_bir_lowering=False, debug=False,
                   num_devices=NCORES)
    f32, bf16 = mybir.dt.float32, mybir.dt.bfloat16

    ins = {}
    for name in ("u0r", "u0i", "pr", "pi", "ccp", "scp"):
        ins[name] = nc.dram_tensor(name, [C, H, W], f32, kind="ExternalInput")
    jout = nc.dram_tensor("jout", [C, H, W], f32, kind="ExternalOutput")

    # DFT matrix F[h,v] = exp(-2i*pi*h*v/N), symmetric.
    idx = np.arange(H, dtype=np.float64)
    ang = -2.0 * np.pi * np.outer(idx, idx) / H
    Fr_np = np.cos(ang).astype(np.float32)
    Fi_np = np.sin(ang).astype(np.float32)

    def chunked(a):  # [512,512] -> [128, 4, 512] with h = 128*j + p
        return np.ascontiguousarray(
            a.reshape(4, 128, W).transpose(1, 0, 2)).astype(ml_dtypes.bfloat16)

    fr_d = nc.inline_tensor(chunked(Fr_np), name="Fr")
    fi_d = nc.inline_tensor(chunked(Fi_np), name="Fi")
    fn_d = nc.inline_tensor(chunked(-Fi_np), name="Fn")

    with tile.TileContext(nc) as tc:
        with (
            tc.tile_pool(name="consts", bufs=1) as consts,
            tc.tile_pool(name="stage", bufs=2) as stage,
            tc.tile_pool(name="plane", bufs=2) as plane,
            tc.tile_pool(name="mid", bufs=1) as mid,
            tc.tile_pool(name="psum", bufs=4, space="PSUM") as psum,
            tc.tile_pool(name="outp", bufs=2) as outp,
        ):
            fr = consts.tile([128, 4, W], bf16)
            fi = consts.tile([128, 4, W], bf16)
            fn = consts.tile([128, 4, W], bf16)
            nc.sync.dma_start(fr[:], fr_d.ap().rearrange("p j w -> p (j w)"))
            nc.sync.dma_start(fi[:], fi_d.ap().rearrange("p j w -> p (j w)"))
            nc.sync.dma_start(fn[:], fn_d.ap().rearrange("p j w -> p (j w)"))
            eps_t = consts.tile([128, 1], f32)
            nc.vector.memset(eps_t[:], 1e-12)

            def load_plane_bf16(dram, c, tag):
                """DMA one [512,512] f32 plane -> bf16 [128,4,512] tile."""
                t32 = stage.tile([128, 4, W], f32, tag="stage32")
                nc.sync.dma_start(
                    t32[:], dram.ap()[c].rearrange("(j p) w -> p j w", p=128))
                t16 = plane.tile([128, 4, W], bf16, tag=tag)
                nc.scalar.copy(t16[:], t32[:])
                return t16

            def dft_pass(ar, ai, rr, ri, rn, tag):
                """out[a,b] = sum_k A[k,a] * R[k,b]  (A = data as lhsT).

                A given as (ar, ai) bf16 [128,4,512]; R = complex rhs with
                components (rr for real-part combo r1, ...) — caller passes
                the four (lhsT, rhs) accumulation pairs explicitly.
                Returns (or_, oi_) bf16 [128,4,512].
                """
                o_r = mid.tile([128, 4, W], bf16, tag=tag + "r")
                o_i = mid.tile([128, 4, W], bf16, tag=tag + "i")
                for m in range(4):
                    ps_r = psum.tile([128, W], f32, tag="psr")
                    ps_i = psum.tile([128, W], f32, tag="psi")
                    for k in range(4):
                        first, last = (k == 0), (k == 3)
                        nc.tensor.matmul(
                            ps_r[:], ar[:, k, bass.ts(m, 128)], rr[:, k, :],
                            start=first, stop=False)
                        nc.tensor.matmul(
                            ps_r[:], ai[:, k, bass.ts(m, 128)], rn[:, k, :],
                            start=False, stop=last)
                        nc.tensor.matmul(
                            ps_i[:], ar[:, k, bass.ts(m, 128)], ri[:, k, :],
                            start=first, stop=False)
                        nc.tensor.matmul(
                            ps_i[:], ai[:, k, bass.ts(m, 128)], rr[:, k, :],
                            start=False, stop=last)
                    nc.any.tensor_copy(o_r[:, m, :], ps_r[:])
                    nc.any.tensor_copy(o_i[:, m, :], ps_i[:])
                return o_r, o_i

            def cmul(ar, ai, br, bi, tag):
                """(ar+i*ai) * (br+i*bi) elementwise -> bf16 tiles."""
                t1 = mid.tile([128, 4, W], bf16, tag="cm_t1")
                t2 = mid.tile([128, 4, W], bf16, tag="cm_t2")
                o_r = plane.tile([128, 4, W], bf16, tag=tag + "r")
                o_i = plane.tile([128, 4, W], bf16, tag=tag + "i")
                nc.vector.tensor_mul(t1[:], ar[:], br[:])
                nc.vector.tensor_mul(t2[:], ai[:], bi[:])
                nc.vector.tensor_sub(o_r[:], t1[:], t2[:])
                nc.vector.tensor_mul(t1[:], ar[:], bi[:])
                nc.vector.tensor_mul(t2[:], ai[:], br[:])
                nc.vector.tensor_add(o_i[:], t1[:], t2[:])
                return o_r, o_i

            for c in range(C):
                xr = load_plane_bf16(ins["u0r"], c, "xr")
                xi = load_plane_bf16(ins["u0i"], c, "xi")

                # forward fft2: two passes against F = Fr + i*Fi
                t1r, t1i = dft_pass(xr, xi, fr, fi, fn, "t1")
                yr, yi = dft_pass(t1r, t1i, fr, fi, fn, "y")

                # multiply by P (includes (1+freq_gain), exp(i kz zm), 1/N^2)
                p_r = load_plane_bf16(ins["pr"], c, "pr")
                p_i = load_plane_bf16(ins["pi"], c, "pi")
                gr, gi = cmul(yr, yi, p_r, p_i, "g")

                # inverse fft2 (unnormalized): two passes against conj(F):
                # real combo: Gr*Fr + Gi*Fi ; imag combo: Gi*Fr - Gr*Fi
                def idft_pass(ar, ai, tag):
                    o_r = mid.tile([128, 4, W], bf16, tag=tag + "r")
                    o_i = mid.tile([128, 4, W], bf16, tag=tag + "i")
                    for m in range(4):
                        ps_r = psum.tile([128, W], f32, tag="psr")
                        ps_i = psum.tile([128, W], f32, tag="psi")
                        for k in range(4):
                            first, last = (k == 0), (k == 3)
                            nc.tensor.matmul(
                                ps_r[:], ar[:, k, bass.ts(m, 128)],
                                fr[:, k, :], start=first, stop=False)
                            nc.tensor.matmul(
                                ps_r[:], ai[:, k, bass.ts(m, 128)],
                                fi[:, k, :], start=False, stop=last)
                            nc.tensor.matmul(
                                ps_i[:], ar[:, k, bass.ts(m, 128)],
                                fn[:, k, :], start=first, stop=False)
                            nc.tensor.matmul(
                                ps_i[:], ai[:, k, bass.ts(m, 128)],
                                fr[:, k, :], start=False, stop=last)
                        nc.any.tensor_copy(o_r[:, m, :], ps_r[:])
                        nc.any.tensor_copy(o_i[:, m, :], ps_i[:])
                    return o_r, o_i

                t3r, t3i = idft_pass(gr, gi, "t3")
                uzr, uzi = idft_pass(t3r, t3i, "uz")

                # phase correction exp(i cp): (uzr+i uzi)*(ccp+i scp)
                c_r = load_plane_bf16(ins["ccp"], c, "ccp")
                c_i = load_plane_bf16(ins["scp"], c, "scp")
                wr, wi = cmul(uzr, uzi, c_r, c_i, "w")

                # J = sqrt(wr^2 + wi^2 + 1e-12)
                sq1 = mid.tile([128, 4, W], bf16, tag="sq1")
                sq2 = mid.tile([128, 4, W], bf16, tag="sq2")
                nc.vector.tensor_mul(sq1[:], wr[:], wr[:])
                nc.vector.tensor_mul(sq2[:], wi[:], wi[:])
                nc.vector.tensor_add(sq1[:], sq1[:], sq2[:])
                jt = outp.tile([128, 4, W], f32, tag="j")
                nc.scalar.activation(jt[:], sq1[:],
                                     mybir.ActivationFunctionType.Sqrt,
                                     bias=eps_t[:], scale=1.0)
                nc.sync.dma_start(
                    jout.ap()[c].rearrange("(j p) w -> p j w", p=128), jt[:])

    nc.compile()
    _KERNEL_CACHE["nc"] = nc
    return nc


# ------------------------------------------------------------------ kernel
def kernel(**inputs):
    x = np.asarray(inputs["x"], np.float32)
    B = x.shape[0]

    u0r, u0i, z, z_mean = _host_pre(
        x, inputs["norm_g"], inputs["norm_b"],
        inputs["ph_w1"], inputs["ph_b1"], inputs["ph_w2"], inputs["ph_b2"],
        inputs["z_w1"], inputs["z_b1"], inputs["z_w2"], inputs["z_b2"])

    # frequency-domain multiplier P = (1+g)/N^2 * exp(i kz z_mean)
    fy = np.fft.fftfreq(H).astype(np.float32)
    fx = np.fft.fftfreq(W).astype(np.float32)
    f2 = fy[:, None] ** 2 + fx[None, :] ** 2
    inv_l2 = (1.0 / WAVELENGTHS ** 2)[:, None, None]
    kz = 2.0 * np.pi * np.sqrt(np.maximum(inv_l2 - f2[None], 0.0))  # (3,H,W)
    gain = (1.0 + np.asarray(inputs["freq_gain"], np.float32))[None, :, None, None]
    hp = kz[None] * z_mean[:, :, :, :]                # (B,3,H,W)
    scale = gain / (H * W)
    pr = (scale * np.cos(hp)).astype(np.float32)
    pi = (scale * np.sin(hp)).astype(np.float32)

    # spatial phase correction exp(i k0 (z - z_mean))
    k0 = (2.0 * np.pi / WAVELENGTHS)[None, :, None, None]
    cp = k0 * (z - z_mean)
    ccp = np.cos(cp).astype(np.float32)
    scp = np.sin(cp).astype(np.float32)

    nc = _build_fft_kernel()
    in_maps = []
    for b in range(NCORES):
        bb = min(b, B - 1)
        in_maps.append({
            "u0r": np.ascontiguousarray(u0r[bb]),
            "u0i": np.ascontiguousarray(u0i[bb]),
            "pr": np.ascontiguousarray(pr[bb]),
            "pi": np.ascontiguousarray(pi[bb]),
            "ccp": np.ascontiguousarray(ccp[bb]),
            "scp": np.ascontiguousarray(scp[bb]),
        })
    global _LAST_IN_MAPS
    _LAST_IN_MAPS = in_maps
    res = run_bass_kernel_spmd(nc, in_maps, core_ids=list(range(NCORES)))
    J = np.stack([res.results[b]["jout"] for b in range(B)], axis=0)

    out = _host_post(
        x, J.astype(np.float32),
        inputs["mix_w1"], inputs["mix_b1"], inputs["gn1_g"], inputs["gn1_b"],
        inputs["mix_w2"], inputs["mix_b2"], inputs["gn2_g"], inputs["gn2_b"],
        inputs["mix_w3"], inputs["mix_b3"],
        inputs["se_w1"], inputs["se_b1"], inputs["se_w2"], inputs["se_b2"],
        np.float32(inputs["alpha"]))
    return np.asarray(out, np.float32)
